# revision 10
# baseline (speedup 1.0000x reference)
"""Distributed Trainium2 (Bass/Tile) kernel for a Qwen3-style attention layer.

Full layer: QKV proj -> per-head RMSNorm (q,k) -> RoPE -> GQA SDPA -> o_proj.

Sharding over 8 NeuronCores:
  - tensor-parallel across heads for QKV+attention: core c owns q-heads
    [4c, 4c+4) and kv-head c; hidden_states replicated.
  - AllToAll exchanges attention context so each core ends with all 4096
    context dims for a 256-token slice; o_proj is then token-parallel with
    Wo replicated (streamed). Output: per-core [256, 4096] chunks that the
    host concatenates. No all-reduce needed.

Compute layout: everything lives transposed ([dim, token]) so the PE array
contracts over the partition axis with N=512 moving tiles in bf16.

Schedule notes (v2):
  - proj streams hidden tt-outer (weights re-read per pass) so first MMs
    start ~3us in and only ~4MB of hidden is resident.
  - RMSNorm rsqrt = exp(-0.5*ln(ms)): every ACT func used (Square/Ln/Exp/
    Copy) lives in the natural_log_exp_and_others table set -> no ~2.7us
    table reloads between softmax exps and norm sqrts.
  - softmax denominator add-tree on GpSimd; q/k half-swap DMAs on the
    GpSimd SWDGE queue; cx gathers + consts on the scalar DMA queue; bulk
    weight/hidden/wo loads + output stores on the sync queue. Keeps the
    collective-dependent cx1 gather from head-of-line blocking wo loads.
  - wo grp0 prefetched before attn1; grp1 partially prefetched mid-attn1;
    o_proj emission: b0:g0, partial b0:g1 fillers, then b1:g0 so the last
    AllToAll + cx1 gather are covered by PE work.
"""

import numpy as np
import ml_dtypes

import concourse.bass as bass
import concourse.mybir as mybir
from concourse import bacc
from concourse.tile import TileContext
from concourse.bass_utils import run_bass_kernel_spmd
from concourse.masks import make_identity

F32 = mybir.dt.float32
BF16 = mybir.dt.bfloat16
BF16_NP = ml_dtypes.bfloat16

N_CORES = 8

FULL_CFG = dict(B=2, S=1024, HID=4096, H=32, KV=8, D=128, eps=1e-6)


def build_program(B=2, S=1024, HID=4096, H=32, KV=8, D=128, eps=1e-6):
    cores = N_CORES
    assert D == 128 and H % cores == 0 and KV == cores and B == 2
    HQ = H // cores            # q heads per core
    HH = HQ // 2               # heads per a2a half (last batch)
    T = B * S                  # total tokens
    HCH = HID // 128           # hidden-dim chunks of 128
    TT = min(512, S)           # projection token tile (within batch)
    TPB = S // TT              # projection tiles per batch
    KB = S // 128              # key blocks per batch
    QT = min(512, S)           # attention q tile
    QTB = S // QT              # q tiles per batch
    TC = T // cores            # output tokens per core
    TCB = TC // B              # per-batch token slice per core
    ICH = (H * D) // 128       # o_proj contraction chunks (32)
    OH = min(512, HID // 2)    # o_proj hid tile width
    NHG = HID // OH            # number of hid groups
    GSZ = 2                    # hid groups per o_proj block
    NG = NHG // GSZ            # o_proj wo groups (4)
    HB = 4                     # hidden chunks per batched DMA tile
    NHT = HCH // HB            # hidden tiles per pass (8)
    WB = 4                     # wo ic-chunks per batched DMA tile
    NWT = ICH // WB            # wo tiles per group (8)
    assert NHG % GSZ == 0 and NG == 4
    scale = float(D) ** -0.5
    MULT = mybir.AluOpType.mult
    SW = QTB * QT              # full q row per batch (== S)

    nc = bacc.Bacc("TRN2", target_bir_lowering=False, debug=False,
                   num_devices=cores)

    hT = nc.dram_tensor("hT", [B, HCH, 128, S], BF16, kind="ExternalInput")
    wq = nc.dram_tensor("wq", [HQ, 128, HCH * 128], BF16, kind="ExternalInput")
    wk = nc.dram_tensor("wk", [128, HCH * 128], BF16, kind="ExternalInput")
    wv = nc.dram_tensor("wv", [128, HCH * 128], BF16, kind="ExternalInput")
    wo = nc.dram_tensor("wo", [ICH, 128, HID], BF16, kind="ExternalInput")
    cosT = nc.dram_tensor("cosT", [128, S], BF16, kind="ExternalInput")
    csinT = nc.dram_tensor("csinT", [128, S], BF16, kind="ExternalInput")
    qw = nc.dram_tensor("qw", [128, 1], F32, kind="ExternalInput")
    kw = nc.dram_tensor("kw", [128, 1], F32, kind="ExternalInput")
    out = nc.dram_tensor("out", [TC, HID], F32, kind="ExternalOutput")

    with TileContext(nc) as tc:
        with (
            tc.tile_pool(name="const", bufs=1) as cp,
            tc.tile_pool(name="dram", bufs=1, space="DRAM") as dramp,
            tc.tile_pool(name="qkv", bufs=1) as p_qkv,
            tc.tile_pool(name="awork", bufs=2) as p_aw,
            tc.tile_pool(name="pt", bufs=2) as p_pt,
            tc.tile_pool(name="psum", bufs=1, space="PSUM") as ps_all,
        ):
            ones_s = cp.tile([128, 128], BF16)
            nc.vector.memset(ones_s[:, :], 1.0)
            ident = cp.tile([128, 128], BF16)
            make_identity(nc, ident[:, :])
            eps_s = cp.tile([128, 1], F32)
            nc.vector.memset(eps_s[:, :], eps)
            cos_s = cp.tile([128, S], BF16)
            nc.scalar.dma_start(out=cos_s[:, :], in_=cosT[:, :])
            csin_s = cp.tile([128, S], BF16)
            nc.scalar.dma_start(out=csin_s[:, :], in_=csinT[:, :])
            qw_s = cp.tile([128, 1], F32)
            nc.scalar.dma_start(out=qw_s[:, :], in_=qw[:, :])
            kw_s = cp.tile([128, 1], F32)
            nc.scalar.dma_start(out=kw_s[:, :], in_=kw[:, :])

            a2a0_in = dramp.tile([H * D, TCB], BF16, name="a2a0i")
            a2a0_out = dramp.tile([H * D, TCB], BF16, name="a2a0o")
            a2a1_in = [dramp.tile([cores * HH * 128, TCB], BF16,
                                  tag=f"a2a1i{p}", name=f"a2a1i{p}")
                       for p in range(2)]
            a2a1_out = [dramp.tile([cores * HH * 128, TCB], BF16,
                                   tag=f"a2a1o{p}", name=f"a2a1o{p}")
                        for p in range(2)]

            qT_s = p_qkv.tile([128, HQ * T], BF16, tag="qT")
            kT_s = p_qkv.tile([128, T], BF16, tag="kT")
            vnat_s = p_qkv.tile([128, T], BF16, tag="vnat")
            ctxT_s = p_qkv.tile([128, HQ * T], BF16, tag="ctxT")

            def rsqrt_act(out_ap, in_ap, bias_ap, sc):
                """rs = rsqrt(in*sc + bias) on ACT.  reciprocal_sqrt lives in
                a table set that also has square/copy, so the whole proj
                phase stays on one set (no ~1.3us reloads per tile)."""
                eng = nc.scalar
                ins = [eng.lower_ap(in_ap), eng.lower_ap(bias_ap),
                       mybir.ImmediateValue(dtype=mybir.dt.float32,
                                            value=float(sc)),
                       mybir.ImmediateValue(dtype=mybir.dt.float32,
                                            value=0.0)]
                return eng.add_instruction(
                    mybir.InstActivation(
                        name=nc.get_next_instruction_name(),
                        func=mybir.ActivationFunctionType.Rsqrt,
                        ins=ins, outs=[eng.lower_ap(out_ap)]))

            def proj(b, wts, p_hid, p_pw, wload_hook=None):
                """QKV projection + norm + rope for batch b (tt-outer)."""
                for tt in range(TPB):
                    tg = b * S + tt * TT
                    pos = tt * TT
                    hts = []
                    for k in range(NHT):
                        t_ = p_hid.tile([128, HB, TT], BF16, tag="hid",
                                        name="hid")
                        nc.sync.dma_start(
                            out=t_[:, :, :],
                            in_=hT[b, k * HB:(k + 1) * HB, :,
                                   tt * TT:(tt + 1) * TT]
                            .rearrange("c p s -> p c s"))
                        hts.append(t_)
                        if wload_hook is not None and tt == 0 and k < HQ + 1:
                            wload_hook(k + 1)
                    for ob in range(HQ + 2):
                        w_t = wts[ob]
                        ps = ps_all.tile([128, TT], F32, tag="mm", name="ps",
                                         bufs=2, padded_shape=[128, SW])
                        for ch in range(HCH):
                            nc.tensor.matmul(
                                ps[:, :],
                                lhsT=w_t[:, ch * 128:(ch + 1) * 128],
                                rhs=hts[ch // HB][:, ch % HB, :],
                                start=(ch == 0), stop=(ch == HCH - 1))
                        if ob <= HQ:
                            is_q = ob < HQ
                            dst = (qT_s[:, ob * T + tg: ob * T + tg + TT]
                                   if is_q else kT_s[:, tg: tg + TT])
                            wcol = qw_s if is_q else kw_s
                            sq = p_pw.tile([128, TT], BF16, tag="sq")
                            nc.scalar.square(sq[:, :], ps[:, :])
                            ssq = ps_all.tile([128, TT], F32, tag="aux",
                                              name="ssq", bufs=2)
                            nc.tensor.matmul(ssq[:, :], lhsT=ones_s[:, :],
                                             rhs=sq[:, :], start=True,
                                             stop=True)
                            rs = p_pw.tile([128, TT], F32, tag="rs")
                            rsqrt_act(rs[:, :], ssq[:, :], eps_s[:, :],
                                      1.0 / D)
                            qn = p_pw.tile([128, TT], F32, tag="qn")
                            nc.vector.scalar_tensor_tensor(
                                qn[:, :], in0=ps[:, :], scalar=wcol[:, :],
                                in1=rs[:, :], op0=MULT, op1=MULT)
                            qsw = p_pw.tile([128, TT], F32, tag="qsw")
                            nc.scalar.dma_start(out=qsw[0:64, :],
                                                in_=qn[64:128, :])
                            nc.scalar.dma_start(out=qsw[64:128, :],
                                                in_=qn[0:64, :])
                            t1 = p_pw.tile([128, TT], F32, tag="t1")
                            nc.vector.tensor_mul(t1[:, :], qn[:, :],
                                                 cos_s[:, pos: pos + TT])
                            t2 = p_pw.tile([128, TT], BF16, tag="t2")
                            nc.vector.tensor_mul(t2[:, :], qsw[:, :],
                                                 csin_s[:, pos: pos + TT])
                            nc.vector.tensor_add(dst, t1[:, :], t2[:, :])
                        else:
                            vt = p_pw.tile([128, TT], BF16, tag="vt")
                            nc.scalar.copy(vt[:, :], ps[:, :])
                            for tb in range(TT // 128):
                                vtr = ps_all.tile([128, 128], BF16, tag="aux",
                                                  name="vtr", bufs=2)
                                nc.tensor.transpose(
                                    vtr[:, :], vt[:, tb * 128:(tb + 1) * 128],
                                    ident[:, :])
                                tbg = tg // 128 + tb
                                nc.scalar.copy(
                                    vnat_s[:, tbg * 128:(tbg + 1) * 128],
                                    vtr[:, :])

            def attn(b, mid_hook=None):
                """Attention for batch b + context shipping (A2A)."""
                last = b == B - 1
                for h in range(HQ):
                    qoff = h * T + b * S
                    pt_t = p_pt.tile([128, KB * SW], BF16, tag="pT",
                                     name="pT")
                    for kb in range(KB):
                        sps = ps_all.tile([128, SW], F32, tag="mm",
                                          name="sps", bufs=2)
                        for qt in range(QTB):
                            nc.tensor.matmul(
                                sps[:, qt * QT:(qt + 1) * QT],
                                lhsT=kT_s[:, b * S + kb * 128:
                                          b * S + (kb + 1) * 128],
                                rhs=qT_s[:, qoff + qt * QT:
                                         qoff + (qt + 1) * QT],
                                start=True, stop=True)
                        nc.scalar.activation(
                            pt_t[:, kb * SW:(kb + 1) * SW], sps[:, :],
                            mybir.ActivationFunctionType.Exp, scale=scale)
                    ctxs = [ps_all.tile([128, QT], F32, tag="ctx", name="ctx",
                                        bufs=2) for _ in range(QTB)]
                    for kb in range(KB):
                        tbg = (b * S) // 128 + kb
                        for qt in range(QTB):
                            nc.tensor.matmul(
                                ctxs[qt][:, :],
                                lhsT=vnat_s[:, tbg * 128:(tbg + 1) * 128],
                                rhs=pt_t[:, kb * SW + qt * QT:
                                         kb * SW + (qt + 1) * QT],
                                start=(kb == 0), stop=(kb == KB - 1))
                    denp = p_aw.tile([128, SW], BF16, tag="denp")
                    dent = p_aw.tile([128, SW], BF16, tag="dent")
                    assert KB % 4 == 0
                    nc.vector.tensor_add(denp[:, :], pt_t[:, 0:SW],
                                         pt_t[:, SW:2 * SW])
                    nc.vector.tensor_add(dent[:, :],
                                         pt_t[:, 2 * SW:3 * SW],
                                         pt_t[:, 3 * SW:4 * SW])
                    nc.vector.tensor_add(denp[:, :], denp[:, :], dent[:, :])
                    for g in range(1, KB // 4):
                        nc.vector.tensor_add(
                            dent[:, :],
                            pt_t[:, 4 * g * SW:(4 * g + 1) * SW],
                            pt_t[:, (4 * g + 1) * SW:(4 * g + 2) * SW])
                        nc.vector.tensor_add(denp[:, :], denp[:, :],
                                             dent[:, :])
                        nc.vector.tensor_add(
                            dent[:, :],
                            pt_t[:, (4 * g + 2) * SW:(4 * g + 3) * SW],
                            pt_t[:, (4 * g + 3) * SW:(4 * g + 4) * SW])
                        nc.vector.tensor_add(denp[:, :], denp[:, :],
                                             dent[:, :])
                    for qt in range(QTB):
                        dps = ps_all.tile([128, QT], F32, tag="aux",
                                          name="dps", bufs=2)
                        nc.tensor.matmul(dps[:, :], lhsT=ones_s[:, :],
                                         rhs=denp[:, qt * QT:(qt + 1) * QT],
                                         start=True, stop=True)
                        rec = p_aw.tile([128, QT], F32, tag="rec")
                        nc.vector.reciprocal_approx_fast(rec[:, :], dps[:, :])
                        nc.vector.tensor_mul(
                            ctxT_s[:, qoff + qt * QT: qoff + (qt + 1) * QT],
                            ctxs[qt][:, :], rec[:, :])
                    # ship this head's context (one batched DMA per head)
                    if not last:
                        nc.gpsimd.dma_start(
                            out=a2a0_in[:, :].rearrange(
                                "(j q p) t -> q p j t", j=cores, p=128)[h],
                            in_=ctxT_s[:, qoff: qoff + S].rearrange(
                                "p (j t) -> p j t", j=cores))
                    else:
                        pi, hh = h // HH, h % HH
                        nc.gpsimd.dma_start(
                            out=a2a1_in[pi][:, :].rearrange(
                                "(j q p) t -> q p j t", j=cores, p=128)[hh],
                            in_=ctxT_s[:, qoff: qoff + S].rearrange(
                                "p (j t) -> p j t", j=cores))
                        if hh == HH - 1:
                            nc.gpsimd.collective_compute(
                                "AllToAll", mybir.AluOpType.bypass,
                                replica_groups=[list(range(cores))],
                                ins=[a2a1_in[pi].opt()],
                                outs=[a2a1_out[pi].opt()])
                            if pi == 0 and mid_hook is not None:
                                mid_hook()
                if not last:
                    nc.gpsimd.collective_compute(
                        "AllToAll", mybir.AluOpType.bypass,
                        replica_groups=[list(range(cores))],
                        ins=[a2a0_in.opt()],
                        outs=[a2a0_out.opt()])

            # ---- phase 1: proj0, attn0 (+a2a0), proj1 ----
            with (
                tc.tile_pool(name="hid", bufs=10) as p_hid,
                tc.tile_pool(name="wts", bufs=6) as p_w,
                tc.tile_pool(name="pwork", bufs=2) as p_pw,
            ):
                # QKV weights: load once, reused for both batches.  Emit
                # interleaved with the first hidden pass so ob=1.. aren't
                # gated behind all of hT.
                wts = {}

                def wload(ob):
                    w_t = p_w.tile([128, HCH * 128], BF16, tag="w", name="w")
                    src = (wq[ob] if ob < HQ else
                           (wk[:, :] if ob == HQ else wv[:, :]))
                    nc.sync.dma_start(out=w_t[:, :], in_=src)
                    wts[ob] = w_t

                wload(0)
                sc_ = nc.enter_named_scope("proj0", True)[0]
                proj(0, wts, p_hid, p_pw, wload_hook=wload)
                nc.leave_named_scope("proj0", sc_, True)
                sc_ = nc.enter_named_scope("attn0", True)[0]
                attn(0)
                nc.leave_named_scope("attn0", sc_, True)
                sc_ = nc.enter_named_scope("proj1", True)[0]
                proj(1, wts, p_hid, p_pw)
                nc.leave_named_scope("proj1", sc_, True)

            # ---- phase 2: attn1 (split a2a) + o_proj ----
            with (
                tc.tile_pool(name="wo", bufs=12) as p_wo,
                tc.tile_pool(name="cx", bufs=1) as p_cx,
                tc.tile_pool(name="oo", bufs=3) as p_oo,
            ):
                wo_tiles = {}   # (grp, k) -> tile

                def load_wo_tile(g, k):
                    w_t = p_wo.tile([128, WB, GSZ * OH], BF16, tag="wo",
                                    name="wo")
                    nc.sync.dma_start(
                        out=w_t[:, :, :],
                        in_=wo[g * 0 + k * WB:(k * WB) + WB, :,
                               g * GSZ * OH:(g + 1) * GSZ * OH]
                        .rearrange("c p w -> p c w"))
                    wo_tiles[(g, k)] = w_t

                def wo_view(g, ic, i):
                    w_t = wo_tiles[(g, ic // WB)]
                    return w_t[:, ic % WB, i * OH:(i + 1) * OH]

                # prefetch grp0 before attn1 (transfers overlap attn1)
                for k in range(NWT):
                    load_wo_tile(0, k)

                cx_s = [p_cx.tile([128, ICH * TCB], BF16, tag=f"cx{b}",
                                  name=f"cx{b}") for b in range(B)]
                # cx0: a2a0 done long ago; scalar queue
                nc.scalar.dma_start(
                    out=cx_s[0][:, :].rearrange("p (ic t) -> p ic t", ic=ICH),
                    in_=a2a0_out[:, :].rearrange("(ic p) t -> p ic t",
                                                 ic=ICH))

                def _mid_hook():
                    for k in range(4):
                        load_wo_tile(1, k)

                sc_ = nc.enter_named_scope("attn1", True)[0]
                attn(1, mid_hook=_mid_hook)
                nc.leave_named_scope("attn1", sc_, True)

                # cx1 gather on scalar queue (waits on a2a1; does not block
                # the sync queue where wo loads live)
                cxv = cx_s[1][:, :].rearrange(
                    "p (j four t) -> p j four t", four=HQ, t=TCB)
                for pi in range(2):
                    srcv = a2a1_out[pi][:, :].rearrange(
                        "(j hh p) t -> p hh j t", hh=HH, p=128)
                    for hh in range(HH):
                        nc.scalar.dma_start(
                            out=cxv[:, :, pi * HH + hh, :],
                            in_=srcv[:, hh])

                sc_ = nc.enter_named_scope("oproj", True)[0]

                def pso_mms(pso_t, b, g, i, ic0, ic1):
                    for ic in range(ic0, ic1):
                        nc.tensor.matmul(
                            pso_t[:, :],
                            lhsT=cx_s[b][:, ic * TCB:(ic + 1) * TCB],
                            rhs=wo_view(g, ic, i),
                            start=(ic == 0), stop=(ic == ICH - 1))

                def finish(pso_t, b, g, i):
                    hg = g * GSZ + i
                    ot = p_oo.tile([TCB, OH], F32, tag="oout", name="oout")
                    nc.vector.tensor_copy(ot[:, :], pso_t[:, :])
                    nc.sync.dma_start(
                        out=out[b * TCB:(b + 1) * TCB,
                                hg * OH:(hg + 1) * OH],
                        in_=ot[:, :])

                def full_block(b, g, i, tag="aux"):
                    pso_t = ps_all.tile([TCB, OH], F32, tag=tag, name="pso",
                                        bufs=2)
                    pso_mms(pso_t, b, g, i, 0, ICH)
                    finish(pso_t, b, g, i)

                # b0:g0 (wo prefetched, cx0 ready)
                full_block(0, 0, 0)
                full_block(0, 0, 1)
                # filler: partial b0:g1 with the 4 prefetched g1 tiles
                pC = ps_all.tile([TCB, OH], F32, tag="ctx", name="psoC",
                                 bufs=2)
                pso_mms(pC, 0, 1, 0, 0, 4 * WB)
                pD = ps_all.tile([TCB, OH], F32, tag="ctx", name="psoD",
                                 bufs=2)
                pso_mms(pD, 0, 1, 1, 0, 4 * WB)
                # b1:g0 (needs cx1 -> covered by the ~29us of work above)
                full_block(1, 0, 0)
                full_block(1, 0, 1)
                # rest of g1 + all of g2 stream into slots freed by b1:g0
                for k in range(4, NWT):
                    load_wo_tile(1, k)
                for k in range(NWT):
                    load_wo_tile(2, k)
                pso_mms(pC, 0, 1, 0, 4 * WB, ICH)
                finish(pC, 0, 1, 0)
                pso_mms(pD, 0, 1, 1, 4 * WB, ICH)
                finish(pD, 0, 1, 1)
                full_block(1, 1, 0)
                full_block(1, 1, 1)
                full_block(0, 2, 0)
                full_block(0, 2, 1)
                for k in range(NWT):
                    load_wo_tile(3, k)
                full_block(1, 2, 0)
                full_block(1, 2, 1)
                for b in range(B):
                    full_block(b, 3, 0)
                    full_block(b, 3, 1)
                nc.leave_named_scope("oproj", sc_, True)

    nc.compile()
    return nc


def host_prep(inputs, B=2, S=1024, HID=4096, H=32, KV=8, D=128, eps=1e-6):
    """Shard + lay out the full inputs into per-core in_maps."""
    cores = N_CORES
    HQ = H // cores
    T = B * S
    HCH = HID // 128
    ICH = (H * D) // 128

    hs = np.ascontiguousarray(inputs["hidden_states"], dtype=np.float32)
    fc = np.asarray(inputs["freqs_cis"], dtype=np.float32)
    Wq = np.asarray(inputs["Wq"], dtype=np.float32)
    Wk = np.asarray(inputs["Wk"], dtype=np.float32)
    Wv = np.asarray(inputs["Wv"], dtype=np.float32)
    Wo = np.asarray(inputs["Wo"], dtype=np.float32)
    qnw = np.asarray(inputs["q_norm_w"], dtype=np.float32)
    knw = np.asarray(inputs["k_norm_w"], dtype=np.float32)

    # hidden^T chunks: hT[b, ch, p, s] = hs[b, s, ch*128+p]
    hT = np.ascontiguousarray(
        hs.transpose(0, 2, 1).reshape(B, HCH, 128, S)).astype(BF16_NP)

    cos, sin, nsin = fc[0], fc[1], fc[2]      # [S, D]
    cosT = np.ascontiguousarray(cos.T).astype(BF16_NP)    # [128, S]
    csinT = np.concatenate([nsin.T[0:64], sin.T[64:128]], axis=0)
    csinT = np.ascontiguousarray(csinT).astype(BF16_NP)
    qw_col = np.ascontiguousarray(qnw.reshape(128, 1))
    kw_col = np.ascontiguousarray(knw.reshape(128, 1))

    # Wo^T chunks: wo[ic, p, hid] = Wo[hid, ic*128+p]
    woT = np.ascontiguousarray(Wo.T.reshape(ICH, 128, HID)).astype(BF16_NP)

    def prep_w(Wm, nblocks):
        # [nblocks, p, ch*128] with w[ob, p, ch*128+j] = Wm[ob*128+j, ch*128+p]
        a = Wm.reshape(nblocks, 128, HCH, 128).transpose(0, 3, 2, 1)
        return np.ascontiguousarray(a.reshape(nblocks, 128, HCH * 128)) \
            .astype(BF16_NP)

    in_maps = []
    for c in range(cores):
        Wq_c = Wq[c * HQ * D:(c + 1) * HQ * D]
        Wk_c = Wk[c * D:(c + 1) * D]
        Wv_c = Wv[c * D:(c + 1) * D]
        in_maps.append({
            "hT": hT,
            "wq": prep_w(Wq_c, HQ),
            "wk": prep_w(Wk_c, 1)[0],
            "wv": prep_w(Wv_c, 1)[0],
            "wo": woT,
            "cosT": cosT,
            "csinT": csinT,
            "qw": qw_col,
            "kw": kw_col,
        })
    return in_maps


def gather_output(results, B=2, S=1024, HID=4096, **_):
    cores = N_CORES
    TCB = (B * S) // cores // B
    out = np.empty((B, S, HID), dtype=np.float32)
    for c in range(cores):
        o = results[c]["out"]
        for b in range(B):
            out[b, c * TCB:(c + 1) * TCB] = o[b * TCB:(b + 1) * TCB]
    return out


_NC_CACHE = {}


def kernel(**inputs) -> np.ndarray:
    cfg = FULL_CFG
    key = tuple(sorted(cfg.items()))
    if key not in _NC_CACHE:
        _NC_CACHE[key] = build_program(**cfg)
    nc = _NC_CACHE[key]
    in_maps = host_prep(inputs, **cfg)
    res = run_bass_kernel_spmd(nc, in_maps, core_ids=list(range(N_CORES)))
    return gather_output(res.results, **cfg)


# revision 12
# speedup vs baseline: 1.0105x; 1.0105x over previous
"""Distributed Trainium2 (Bass/Tile) kernel for a Qwen3-style attention layer.

Full layer: QKV proj -> per-head RMSNorm (q,k) -> RoPE -> GQA SDPA -> o_proj.

Sharding over 8 NeuronCores:
  - tensor-parallel across heads for QKV+attention: core c owns q-heads
    [4c, 4c+4) and kv-head c; hidden_states replicated.
  - AllToAll exchanges attention context so each core ends with all 4096
    context dims for a 256-token slice; o_proj is then token-parallel with
    Wo replicated (streamed). Output: per-core [256, 4096] chunks that the
    host concatenates. No all-reduce needed.

Compute layout: everything lives transposed ([dim, token]) so the PE array
contracts over the partition axis with N=512 moving tiles in bf16.

Schedule notes (v2):
  - proj streams hidden tt-outer (weights re-read per pass) so first MMs
    start ~3us in and only ~4MB of hidden is resident.
  - RMSNorm rsqrt = exp(-0.5*ln(ms)): every ACT func used (Square/Ln/Exp/
    Copy) lives in the natural_log_exp_and_others table set -> no ~2.7us
    table reloads between softmax exps and norm sqrts.
  - softmax denominator add-tree on GpSimd; q/k half-swap DMAs on the
    GpSimd SWDGE queue; cx gathers + consts on the scalar DMA queue; bulk
    weight/hidden/wo loads + output stores on the sync queue. Keeps the
    collective-dependent cx1 gather from head-of-line blocking wo loads.
  - wo grp0 prefetched before attn1; grp1 partially prefetched mid-attn1;
    o_proj emission: b0:g0, partial b0:g1 fillers, then b1:g0 so the last
    AllToAll + cx1 gather are covered by PE work.
"""

import numpy as np
import ml_dtypes

import concourse.bass as bass
import concourse.mybir as mybir
from concourse import bacc
from concourse.tile import TileContext
from concourse.bass_utils import run_bass_kernel_spmd
from concourse.masks import make_identity

F32 = mybir.dt.float32
BF16 = mybir.dt.bfloat16
BF16_NP = ml_dtypes.bfloat16

N_CORES = 8

FULL_CFG = dict(B=2, S=1024, HID=4096, H=32, KV=8, D=128, eps=1e-6)


def build_program(B=2, S=1024, HID=4096, H=32, KV=8, D=128, eps=1e-6):
    cores = N_CORES
    assert D == 128 and H % cores == 0 and KV == cores and B == 2
    HQ = H // cores            # q heads per core
    HH = HQ // 2               # heads per a2a half (last batch)
    T = B * S                  # total tokens
    HCH = HID // 128           # hidden-dim chunks of 128
    TT = min(512, S)           # projection token tile (within batch)
    TPB = S // TT              # projection tiles per batch
    KB = S // 128              # key blocks per batch
    QT = min(512, S)           # attention q tile
    QTB = S // QT              # q tiles per batch
    TC = T // cores            # output tokens per core
    TCB = TC // B              # per-batch token slice per core
    ICH = (H * D) // 128       # o_proj contraction chunks (32)
    OH = min(512, HID // 2)    # o_proj hid tile width
    NHG = HID // OH            # number of hid groups
    GSZ = 2                    # hid groups per o_proj block
    NG = NHG // GSZ            # o_proj wo groups (4)
    HB = 4                     # hidden chunks per batched DMA tile
    NHT = HCH // HB            # hidden tiles per pass (8)
    WB = 4                     # wo ic-chunks per batched DMA tile
    NWT = ICH // WB            # wo tiles per group (8)
    assert NHG % GSZ == 0 and NG == 4
    scale = float(D) ** -0.5
    MULT = mybir.AluOpType.mult
    SW = QTB * QT              # full q row per batch (== S)

    nc = bacc.Bacc("TRN2", target_bir_lowering=False, debug=False,
                   num_devices=cores)

    hT = nc.dram_tensor("hT", [B, HCH, 128, S], BF16, kind="ExternalInput")
    wq = nc.dram_tensor("wq", [HQ, 128, HCH * 128], BF16, kind="ExternalInput")
    wk = nc.dram_tensor("wk", [128, HCH * 128], BF16, kind="ExternalInput")
    wv = nc.dram_tensor("wv", [128, HCH * 128], BF16, kind="ExternalInput")
    wo = nc.dram_tensor("wo", [ICH, 128, HID], BF16, kind="ExternalInput")
    cosT = nc.dram_tensor("cosT", [128, S], BF16, kind="ExternalInput")
    csinT = nc.dram_tensor("csinT", [128, S], BF16, kind="ExternalInput")
    qw = nc.dram_tensor("qw", [128, 1], F32, kind="ExternalInput")
    kw = nc.dram_tensor("kw", [128, 1], F32, kind="ExternalInput")
    out = nc.dram_tensor("out", [TC, HID], F32, kind="ExternalOutput")

    with TileContext(nc) as tc:
        with (
            tc.tile_pool(name="const", bufs=1) as cp,
            tc.tile_pool(name="dram", bufs=1, space="DRAM") as dramp,
            tc.tile_pool(name="qkv", bufs=1) as p_qkv,
            tc.tile_pool(name="awork", bufs=2) as p_aw,
            tc.tile_pool(name="pt", bufs=2) as p_pt,
            tc.tile_pool(name="psum", bufs=1, space="PSUM") as ps_all,
        ):
            ones_s = cp.tile([128, 128], BF16)
            nc.vector.memset(ones_s[:, :], 1.0)
            ident = cp.tile([128, 128], BF16)
            make_identity(nc, ident[:, :])
            eps_s = cp.tile([128, 1], F32)
            nc.vector.memset(eps_s[:, :], eps)
            cos_s = cp.tile([128, S], BF16)
            nc.scalar.dma_start(out=cos_s[:, :], in_=cosT[:, :])
            csin_s = cp.tile([128, S], BF16)
            nc.scalar.dma_start(out=csin_s[:, :], in_=csinT[:, :])
            qw_s = cp.tile([128, 1], F32)
            nc.scalar.dma_start(out=qw_s[:, :], in_=qw[:, :])
            kw_s = cp.tile([128, 1], F32)
            nc.scalar.dma_start(out=kw_s[:, :], in_=kw[:, :])

            a2a0_in = dramp.tile([H * D, TCB], BF16, name="a2a0i")
            a2a0_out = dramp.tile([H * D, TCB], BF16, name="a2a0o")
            a2a1_in = [dramp.tile([cores * HH * 128, TCB], BF16,
                                  tag=f"a2a1i{p}", name=f"a2a1i{p}")
                       for p in range(2)]
            a2a1_out = [dramp.tile([cores * HH * 128, TCB], BF16,
                                   tag=f"a2a1o{p}", name=f"a2a1o{p}")
                        for p in range(2)]

            qT_s = p_qkv.tile([128, HQ * T], BF16, tag="qT")
            kT_s = p_qkv.tile([128, T], BF16, tag="kT")
            vnat_s = p_qkv.tile([128, T], BF16, tag="vnat")
            ctxT_s = p_qkv.tile([128, HQ * T], BF16, tag="ctxT")

            def rsqrt_act(out_ap, in_ap, bias_ap, sc):
                """rs = rsqrt(in*sc + bias) on ACT.  reciprocal_sqrt lives in
                a table set that also has square/copy, so the whole proj
                phase stays on one set (no ~1.3us reloads per tile)."""
                eng = nc.scalar
                ins = [eng.lower_ap(in_ap), eng.lower_ap(bias_ap),
                       mybir.ImmediateValue(dtype=mybir.dt.float32,
                                            value=float(sc)),
                       mybir.ImmediateValue(dtype=mybir.dt.float32,
                                            value=0.0)]
                return eng.add_instruction(
                    mybir.InstActivation(
                        name=nc.get_next_instruction_name(),
                        func=mybir.ActivationFunctionType.Rsqrt,
                        ins=ins, outs=[eng.lower_ap(out_ap)]))

            def proj(b, wts, p_hid, p_pw, wload_hook=None):
                """QKV projection + norm + rope for batch b (tt-outer)."""
                for tt in range(TPB):
                    tg = b * S + tt * TT
                    pos = tt * TT
                    hts = []
                    for k in range(NHT):
                        t_ = p_hid.tile([128, HB, TT], BF16, tag="hid",
                                        name="hid")
                        nc.sync.dma_start(
                            out=t_[:, :, :],
                            in_=hT[b, k * HB:(k + 1) * HB, :,
                                   tt * TT:(tt + 1) * TT]
                            .rearrange("c p s -> p c s"))
                        hts.append(t_)
                        if wload_hook is not None and tt == 0 and k < HQ + 1:
                            wload_hook(k + 1)
                    for ob in range(HQ + 2):
                        w_t = wts[ob]
                        ps = ps_all.tile([128, TT], F32, tag="mm", name="ps",
                                         bufs=2, padded_shape=[128, SW])
                        for ch in range(HCH):
                            nc.tensor.matmul(
                                ps[:, :],
                                lhsT=w_t[:, ch * 128:(ch + 1) * 128],
                                rhs=hts[ch // HB][:, ch % HB, :],
                                start=(ch == 0), stop=(ch == HCH - 1))
                        if ob <= HQ:
                            is_q = ob < HQ
                            dst = (qT_s[:, ob * T + tg: ob * T + tg + TT]
                                   if is_q else kT_s[:, tg: tg + TT])
                            wcol = qw_s if is_q else kw_s
                            sq = p_pw.tile([128, TT], BF16, tag="sq")
                            nc.scalar.square(sq[:, :], ps[:, :])
                            ssq = ps_all.tile([128, TT], F32, tag="aux",
                                              name="ssq", bufs=2)
                            nc.tensor.matmul(ssq[:, :], lhsT=ones_s[:, :],
                                             rhs=sq[:, :], start=True,
                                             stop=True)
                            rs = p_pw.tile([128, TT], F32, tag="rs")
                            rsqrt_act(rs[:, :], ssq[:, :], eps_s[:, :],
                                      1.0 / D)
                            qn = p_pw.tile([128, TT], F32, tag="qn")
                            nc.vector.scalar_tensor_tensor(
                                qn[:, :], in0=ps[:, :], scalar=wcol[:, :],
                                in1=rs[:, :], op0=MULT, op1=MULT)
                            qsw = p_pw.tile([128, TT], F32, tag="qsw")
                            nc.scalar.dma_start(out=qsw[0:64, :],
                                                in_=qn[64:128, :])
                            nc.scalar.dma_start(out=qsw[64:128, :],
                                                in_=qn[0:64, :])
                            t1 = p_pw.tile([128, TT], F32, tag="t1")
                            nc.vector.tensor_mul(t1[:, :], qn[:, :],
                                                 cos_s[:, pos: pos + TT])
                            t2 = p_pw.tile([128, TT], BF16, tag="t2")
                            nc.vector.tensor_mul(t2[:, :], qsw[:, :],
                                                 csin_s[:, pos: pos + TT])
                            nc.vector.tensor_add(dst, t1[:, :], t2[:, :])
                        else:
                            vt = p_pw.tile([128, TT], BF16, tag="vt")
                            nc.scalar.copy(vt[:, :], ps[:, :])
                            for tb in range(TT // 128):
                                vtr = ps_all.tile([128, 128], BF16, tag="aux",
                                                  name="vtr", bufs=2)
                                nc.tensor.transpose(
                                    vtr[:, :], vt[:, tb * 128:(tb + 1) * 128],
                                    ident[:, :])
                                tbg = tg // 128 + tb
                                nc.scalar.copy(
                                    vnat_s[:, tbg * 128:(tbg + 1) * 128],
                                    vtr[:, :])

            def attn(b, mid_hook=None):
                """Attention for batch b + context shipping (A2A)."""
                last = b == B - 1
                for h in range(HQ):
                    qoff = h * T + b * S
                    pt_t = p_pt.tile([128, KB * SW], BF16, tag="pT",
                                     name="pT")
                    for kb in range(KB):
                        sps = ps_all.tile([128, SW], F32, tag="mm",
                                          name="sps", bufs=2)
                        for qt in range(QTB):
                            nc.tensor.matmul(
                                sps[:, qt * QT:(qt + 1) * QT],
                                lhsT=kT_s[:, b * S + kb * 128:
                                          b * S + (kb + 1) * 128],
                                rhs=qT_s[:, qoff + qt * QT:
                                         qoff + (qt + 1) * QT],
                                start=True, stop=True)
                        nc.scalar.activation(
                            pt_t[:, kb * SW:(kb + 1) * SW], sps[:, :],
                            mybir.ActivationFunctionType.Exp, scale=scale)
                    ctxs = [ps_all.tile([128, QT], F32, tag="ctx", name="ctx",
                                        bufs=2) for _ in range(QTB)]
                    for kb in range(KB):
                        tbg = (b * S) // 128 + kb
                        for qt in range(QTB):
                            nc.tensor.matmul(
                                ctxs[qt][:, :],
                                lhsT=vnat_s[:, tbg * 128:(tbg + 1) * 128],
                                rhs=pt_t[:, kb * SW + qt * QT:
                                         kb * SW + (qt + 1) * QT],
                                start=(kb == 0), stop=(kb == KB - 1))
                    denp = p_aw.tile([128, SW], BF16, tag="denp")
                    dent = p_aw.tile([128, SW], BF16, tag="dent")
                    assert KB % 4 == 0
                    nc.vector.tensor_add(denp[:, :], pt_t[:, 0:SW],
                                         pt_t[:, SW:2 * SW])
                    nc.vector.tensor_add(dent[:, :],
                                         pt_t[:, 2 * SW:3 * SW],
                                         pt_t[:, 3 * SW:4 * SW])
                    nc.vector.tensor_add(denp[:, :], denp[:, :], dent[:, :])
                    for g in range(1, KB // 4):
                        nc.vector.tensor_add(
                            dent[:, :],
                            pt_t[:, 4 * g * SW:(4 * g + 1) * SW],
                            pt_t[:, (4 * g + 1) * SW:(4 * g + 2) * SW])
                        nc.vector.tensor_add(denp[:, :], denp[:, :],
                                             dent[:, :])
                        nc.vector.tensor_add(
                            dent[:, :],
                            pt_t[:, (4 * g + 2) * SW:(4 * g + 3) * SW],
                            pt_t[:, (4 * g + 3) * SW:(4 * g + 4) * SW])
                        nc.vector.tensor_add(denp[:, :], denp[:, :],
                                             dent[:, :])
                    for qt in range(QTB):
                        dps = ps_all.tile([128, QT], F32, tag="aux",
                                          name="dps", bufs=2)
                        nc.tensor.matmul(dps[:, :], lhsT=ones_s[:, :],
                                         rhs=denp[:, qt * QT:(qt + 1) * QT],
                                         start=True, stop=True)
                        rec = p_aw.tile([128, QT], F32, tag="rec")
                        nc.vector.reciprocal_approx_fast(rec[:, :], dps[:, :])
                        nc.vector.tensor_mul(
                            ctxT_s[:, qoff + qt * QT: qoff + (qt + 1) * QT],
                            ctxs[qt][:, :], rec[:, :])
                    # ship this head's context (one batched DMA per head)
                    if not last:
                        nc.gpsimd.dma_start(
                            out=a2a0_in[:, :].rearrange(
                                "(j q p) t -> q p j t", j=cores, p=128)[h],
                            in_=ctxT_s[:, qoff: qoff + S].rearrange(
                                "p (j t) -> p j t", j=cores))
                    else:
                        pi, hh = h // HH, h % HH
                        nc.gpsimd.dma_start(
                            out=a2a1_in[pi][:, :].rearrange(
                                "(j q p) t -> q p j t", j=cores, p=128)[hh],
                            in_=ctxT_s[:, qoff: qoff + S].rearrange(
                                "p (j t) -> p j t", j=cores))
                        if hh == HH - 1:
                            nc.gpsimd.collective_compute(
                                "AllToAll", mybir.AluOpType.bypass,
                                replica_groups=[list(range(cores))],
                                ins=[a2a1_in[pi].opt()],
                                outs=[a2a1_out[pi].opt()])
                            if pi == 0 and mid_hook is not None:
                                mid_hook()
                if not last:
                    nc.gpsimd.collective_compute(
                        "AllToAll", mybir.AluOpType.bypass,
                        replica_groups=[list(range(cores))],
                        ins=[a2a0_in.opt()],
                        outs=[a2a0_out.opt()])

            # ---- phase 1: proj0, attn0 (+a2a0), proj1 ----
            with (
                tc.tile_pool(name="hid", bufs=10) as p_hid,
                tc.tile_pool(name="wts", bufs=6) as p_w,
                tc.tile_pool(name="pwork", bufs=2) as p_pw,
            ):
                # QKV weights: load once, reused for both batches.  Emit
                # interleaved with the first hidden pass so ob=1.. aren't
                # gated behind all of hT.
                wts = {}

                def wload(ob):
                    w_t = p_w.tile([128, HCH * 128], BF16, tag="w", name="w")
                    src = (wq[ob] if ob < HQ else
                           (wk[:, :] if ob == HQ else wv[:, :]))
                    nc.sync.dma_start(out=w_t[:, :], in_=src)
                    wts[ob] = w_t

                wload(0)
                sc_ = nc.enter_named_scope("proj0", True)[0]
                proj(0, wts, p_hid, p_pw, wload_hook=wload)
                nc.leave_named_scope("proj0", sc_, True)
                sc_ = nc.enter_named_scope("attn0", True)[0]
                attn(0)
                nc.leave_named_scope("attn0", sc_, True)
                sc_ = nc.enter_named_scope("proj1", True)[0]
                proj(1, wts, p_hid, p_pw)
                nc.leave_named_scope("proj1", sc_, True)

            # ---- phase 2: attn1 (split a2a) + o_proj ----
            with (
                tc.tile_pool(name="wo", bufs=11) as p_wo,
                tc.tile_pool(name="cx", bufs=1) as p_cx,
                tc.tile_pool(name="oo", bufs=4) as p_oo,
            ):
                wo_tiles = {}   # (grp, k) -> tile

                def load_wo_tile(g, k):
                    w_t = p_wo.tile([128, WB, GSZ * OH], BF16, tag="wo",
                                    name="wo")
                    nc.sync.dma_start(
                        out=w_t[:, :, :],
                        in_=wo[g * 0 + k * WB:(k * WB) + WB, :,
                               g * GSZ * OH:(g + 1) * GSZ * OH]
                        .rearrange("c p w -> p c w"))
                    wo_tiles[(g, k)] = w_t

                def wo_view(g, ic, i):
                    w_t = wo_tiles[(g, ic // WB)]
                    return w_t[:, ic % WB, i * OH:(i + 1) * OH]

                # prefetch grp0 before attn1 (transfers overlap attn1)
                for k in range(NWT):
                    load_wo_tile(0, k)

                cx0_s = p_cx.tile([128, ICH * TCB], BF16, tag="cx0",
                                  name="cx0")
                # cx1 split per a2a half so b1 o_proj can start on the
                # first half while the second collective is still in flight
                cx1h = [p_cx.tile([128, (ICH // 2) * TCB], BF16,
                                  tag=f"cx1h{p}", name=f"cx1h{p}")
                        for p in range(2)]
                # cx0: a2a0 done long ago; scalar queue
                nc.scalar.dma_start(
                    out=cx0_s[:, :].rearrange("p (ic t) -> p ic t", ic=ICH),
                    in_=a2a0_out[:, :].rearrange("(ic p) t -> p ic t",
                                                 ic=ICH))

                def _mid_hook():
                    for k in range(4):
                        load_wo_tile(1, k)

                def cx_ap(b, ic):
                    if b == 0:
                        return cx0_s[:, ic * TCB:(ic + 1) * TCB]
                    # ic = global head = j * HQ + (pi * HH + hh)
                    q = ic % HQ
                    j = ic // HQ
                    pi, hh = q // HH, q % HH
                    off = (hh * cores + j) * TCB
                    return cx1h[pi][:, off: off + TCB]

                # b=1 accumulation order: pi0's chunks first (they land
                # ~20us before pi1's)
                IC_B1 = [j * HQ + pi * HH + hh
                         for pi in range(2) for hh in range(HH)
                         for j in range(cores)]

                sc_ = nc.enter_named_scope("attn1", True)[0]
                attn(1, mid_hook=_mid_hook)
                nc.leave_named_scope("attn1", sc_, True)

                # cx1 gather on scalar queue (waits on a2a1; does not block
                # the sync queue where wo loads live)
                for pi in range(2):
                    srcv = a2a1_out[pi][:, :].rearrange(
                        "(j hh p) t -> p hh j t", hh=HH, p=128)
                    dstv = cx1h[pi][:, :].rearrange(
                        "p (hh j t) -> p hh j t", hh=HH, j=cores)
                    for hh in range(HH):
                        nc.scalar.dma_start(
                            out=dstv[:, hh],
                            in_=srcv[:, hh])

                sc_ = nc.enter_named_scope("oproj", True)[0]

                def pso_mms(pso_t, b, g, i, k0, k1):
                    order = IC_B1 if b == 1 else list(range(ICH))
                    for kk in range(k0, k1):
                        ic = order[kk]
                        nc.tensor.matmul(
                            pso_t[:, :],
                            lhsT=cx_ap(b, ic),
                            rhs=wo_view(g, ic, i),
                            start=(kk == 0), stop=(kk == ICH - 1))

                def finish(pso_t, b, g, i):
                    hg = g * GSZ + i
                    ot = p_oo.tile([TCB, OH], F32, tag="oout", name="oout")
                    nc.vector.tensor_copy(ot[:, :], pso_t[:, :])
                    nc.sync.dma_start(
                        out=out[b * TCB:(b + 1) * TCB,
                                hg * OH:(hg + 1) * OH],
                        in_=ot[:, :])

                def full_block(b, g, i, tag="aux"):
                    pso_t = ps_all.tile([TCB, OH], F32, tag=tag, name="pso",
                                        bufs=2)
                    pso_mms(pso_t, b, g, i, 0, ICH)
                    finish(pso_t, b, g, i)

                # b0:g0 (wo prefetched, cx0 ready)
                full_block(0, 0, 0)
                full_block(0, 0, 1)
                # filler: partial b0:g1 with the 4 prefetched g1 tiles
                pC = ps_all.tile([TCB, OH], F32, tag="ctx", name="psoC",
                                 bufs=2)
                pso_mms(pC, 0, 1, 0, 0, 4 * WB)
                pD = ps_all.tile([TCB, OH], F32, tag="ctx", name="psoD",
                                 bufs=2)
                pso_mms(pD, 0, 1, 1, 0, 4 * WB)
                # b1:g0 half-interleaved: first halves need only the
                # pi0 a2a half (lands ~20us before pi1)
                pE = ps_all.tile([TCB, OH], F32, tag="aux", name="psoE",
                                 bufs=2)
                pF = ps_all.tile([TCB, OH], F32, tag="aux", name="psoF",
                                 bufs=2)
                pso_mms(pE, 1, 0, 0, 0, ICH // 2)
                pso_mms(pF, 1, 0, 1, 0, ICH // 2)
                pso_mms(pE, 1, 0, 0, ICH // 2, ICH)
                finish(pE, 1, 0, 0)
                pso_mms(pF, 1, 0, 1, ICH // 2, ICH)
                finish(pF, 1, 0, 1)
                # rest of g1 + all of g2 stream into slots freed by b1:g0
                for k in range(4, NWT):
                    load_wo_tile(1, k)
                for k in range(NWT):
                    load_wo_tile(2, k)
                pso_mms(pC, 0, 1, 0, 4 * WB, ICH)
                finish(pC, 0, 1, 0)
                pso_mms(pD, 0, 1, 1, 4 * WB, ICH)
                finish(pD, 0, 1, 1)
                full_block(1, 1, 0)
                full_block(1, 1, 1)
                full_block(0, 2, 0)
                full_block(0, 2, 1)
                for k in range(NWT):
                    load_wo_tile(3, k)
                full_block(1, 2, 0)
                full_block(1, 2, 1)
                for b in range(B):
                    full_block(b, 3, 0)
                    full_block(b, 3, 1)
                nc.leave_named_scope("oproj", sc_, True)

    nc.compile()
    return nc


def host_prep(inputs, B=2, S=1024, HID=4096, H=32, KV=8, D=128, eps=1e-6):
    """Shard + lay out the full inputs into per-core in_maps."""
    cores = N_CORES
    HQ = H // cores
    T = B * S
    HCH = HID // 128
    ICH = (H * D) // 128

    hs = np.ascontiguousarray(inputs["hidden_states"], dtype=np.float32)
    fc = np.asarray(inputs["freqs_cis"], dtype=np.float32)
    Wq = np.asarray(inputs["Wq"], dtype=np.float32)
    Wk = np.asarray(inputs["Wk"], dtype=np.float32)
    Wv = np.asarray(inputs["Wv"], dtype=np.float32)
    Wo = np.asarray(inputs["Wo"], dtype=np.float32)
    qnw = np.asarray(inputs["q_norm_w"], dtype=np.float32)
    knw = np.asarray(inputs["k_norm_w"], dtype=np.float32)

    # hidden^T chunks: hT[b, ch, p, s] = hs[b, s, ch*128+p]
    hT = np.ascontiguousarray(
        hs.transpose(0, 2, 1).reshape(B, HCH, 128, S)).astype(BF16_NP)

    cos, sin, nsin = fc[0], fc[1], fc[2]      # [S, D]
    cosT = np.ascontiguousarray(cos.T).astype(BF16_NP)    # [128, S]
    csinT = np.concatenate([nsin.T[0:64], sin.T[64:128]], axis=0)
    csinT = np.ascontiguousarray(csinT).astype(BF16_NP)
    qw_col = np.ascontiguousarray(qnw.reshape(128, 1))
    kw_col = np.ascontiguousarray(knw.reshape(128, 1))

    # Wo^T chunks: wo[ic, p, hid] = Wo[hid, ic*128+p]
    woT = np.ascontiguousarray(Wo.T.reshape(ICH, 128, HID)).astype(BF16_NP)

    def prep_w(Wm, nblocks):
        # [nblocks, p, ch*128] with w[ob, p, ch*128+j] = Wm[ob*128+j, ch*128+p]
        a = Wm.reshape(nblocks, 128, HCH, 128).transpose(0, 3, 2, 1)
        return np.ascontiguousarray(a.reshape(nblocks, 128, HCH * 128)) \
            .astype(BF16_NP)

    in_maps = []
    for c in range(cores):
        Wq_c = Wq[c * HQ * D:(c + 1) * HQ * D]
        Wk_c = Wk[c * D:(c + 1) * D]
        Wv_c = Wv[c * D:(c + 1) * D]
        in_maps.append({
            "hT": hT,
            "wq": prep_w(Wq_c, HQ),
            "wk": prep_w(Wk_c, 1)[0],
            "wv": prep_w(Wv_c, 1)[0],
            "wo": woT,
            "cosT": cosT,
            "csinT": csinT,
            "qw": qw_col,
            "kw": kw_col,
        })
    return in_maps


def gather_output(results, B=2, S=1024, HID=4096, **_):
    cores = N_CORES
    TCB = (B * S) // cores // B
    out = np.empty((B, S, HID), dtype=np.float32)
    for c in range(cores):
        o = results[c]["out"]
        for b in range(B):
            out[b, c * TCB:(c + 1) * TCB] = o[b * TCB:(b + 1) * TCB]
    return out


_NC_CACHE = {}


def kernel(**inputs) -> np.ndarray:
    cfg = FULL_CFG
    key = tuple(sorted(cfg.items()))
    if key not in _NC_CACHE:
        _NC_CACHE[key] = build_program(**cfg)
    nc = _NC_CACHE[key]
    in_maps = host_prep(inputs, **cfg)
    res = run_bass_kernel_spmd(nc, in_maps, core_ids=list(range(N_CORES)))
    return gather_output(res.results, **cfg)


# revision 13
# speedup vs baseline: 1.0360x; 1.0253x over previous
"""Distributed Trainium2 (Bass/Tile) kernel for a Qwen3-style attention layer.

Full layer: QKV proj -> per-head RMSNorm (q,k) -> RoPE -> GQA SDPA -> o_proj.

Sharding over 8 NeuronCores:
  - tensor-parallel across heads for QKV+attention: core c owns q-heads
    [4c, 4c+4) and kv-head c; hidden_states replicated.
  - AllToAll exchanges attention context so each core ends with all 4096
    context dims for a 256-token slice; o_proj is then token-parallel with
    Wo replicated (streamed). Output: per-core [256, 4096] chunks that the
    host concatenates. No all-reduce needed.

Compute layout: everything lives transposed ([dim, token]) so the PE array
contracts over the partition axis with N=512 moving tiles in bf16.

Schedule notes (v2):
  - proj streams hidden tt-outer (weights re-read per pass) so first MMs
    start ~3us in and only ~4MB of hidden is resident.
  - RMSNorm rsqrt = exp(-0.5*ln(ms)): every ACT func used (Square/Ln/Exp/
    Copy) lives in the natural_log_exp_and_others table set -> no ~2.7us
    table reloads between softmax exps and norm sqrts.
  - softmax denominator add-tree on GpSimd; q/k half-swap DMAs on the
    GpSimd SWDGE queue; cx gathers + consts on the scalar DMA queue; bulk
    weight/hidden/wo loads + output stores on the sync queue. Keeps the
    collective-dependent cx1 gather from head-of-line blocking wo loads.
  - wo grp0 prefetched before attn1; grp1 partially prefetched mid-attn1;
    o_proj emission: b0:g0, partial b0:g1 fillers, then b1:g0 so the last
    AllToAll + cx1 gather are covered by PE work.
"""

import numpy as np
import ml_dtypes

import concourse.bass as bass
import concourse.mybir as mybir
from concourse import bacc
from concourse.tile import TileContext
from concourse.bass_utils import run_bass_kernel_spmd
from concourse.masks import make_identity

F32 = mybir.dt.float32
BF16 = mybir.dt.bfloat16
BF16_NP = ml_dtypes.bfloat16

N_CORES = 8

FULL_CFG = dict(B=2, S=1024, HID=4096, H=32, KV=8, D=128, eps=1e-6)


def build_program(B=2, S=1024, HID=4096, H=32, KV=8, D=128, eps=1e-6):
    cores = N_CORES
    assert D == 128 and H % cores == 0 and KV == cores and B == 2
    HQ = H // cores            # q heads per core
    HH = HQ // 2               # heads per a2a half (last batch)
    T = B * S                  # total tokens
    HCH = HID // 128           # hidden-dim chunks of 128
    TT = min(512, S)           # projection token tile (within batch)
    TPB = S // TT              # projection tiles per batch
    KB = S // 128              # key blocks per batch
    QT = min(512, S)           # attention q tile
    QTB = S // QT              # q tiles per batch
    TC = T // cores            # output tokens per core
    TCB = TC // B              # per-batch token slice per core
    ICH = (H * D) // 128       # o_proj contraction chunks (32)
    OH = min(512, HID // 2)    # o_proj hid tile width
    NHG = HID // OH            # number of hid groups
    GSZ = 2                    # hid groups per o_proj block
    NG = NHG // GSZ            # o_proj wo groups (4)
    HB = 4                     # hidden chunks per batched DMA tile
    NHT = HCH // HB            # hidden tiles per pass (8)
    WB = 4                     # wo ic-chunks per batched DMA tile
    NWT = ICH // WB            # wo tiles per group (8)
    assert NHG % GSZ == 0 and NG == 4
    scale = float(D) ** -0.5
    MULT = mybir.AluOpType.mult
    SW = QTB * QT              # full q row per batch (== S)

    nc = bacc.Bacc("TRN2", target_bir_lowering=False, debug=False,
                   num_devices=cores)

    hT = nc.dram_tensor("hT", [B, TPB, HCH, 128, TT], BF16,
                        kind="ExternalInput")
    wq = nc.dram_tensor("wq", [HQ, 128, HCH * 128], BF16, kind="ExternalInput")
    wk = nc.dram_tensor("wk", [128, HCH * 128], BF16, kind="ExternalInput")
    wv = nc.dram_tensor("wv", [128, HCH * 128], BF16, kind="ExternalInput")
    wo = nc.dram_tensor("wo", [NG, ICH, 128, GSZ * OH], BF16,
                        kind="ExternalInput")
    cosT = nc.dram_tensor("cosT", [128, S], BF16, kind="ExternalInput")
    csinT = nc.dram_tensor("csinT", [128, S], BF16, kind="ExternalInput")
    qw = nc.dram_tensor("qw", [128, 1], F32, kind="ExternalInput")
    kw = nc.dram_tensor("kw", [128, 1], F32, kind="ExternalInput")
    out = nc.dram_tensor("out", [TC, HID], F32, kind="ExternalOutput")

    with TileContext(nc) as tc:
        with (
            tc.tile_pool(name="const", bufs=1) as cp,
            tc.tile_pool(name="dram", bufs=1, space="DRAM") as dramp,
            tc.tile_pool(name="qkv", bufs=1) as p_qkv,
            tc.tile_pool(name="awork", bufs=2) as p_aw,
            tc.tile_pool(name="pt", bufs=2) as p_pt,
            tc.tile_pool(name="psum", bufs=1, space="PSUM") as ps_all,
        ):
            ones_s = cp.tile([128, 128], BF16)
            nc.vector.memset(ones_s[:, :], 1.0)
            ident = cp.tile([128, 128], BF16)
            make_identity(nc, ident[:, :])
            eps_s = cp.tile([128, 1], F32)
            nc.vector.memset(eps_s[:, :], eps)
            cos_s = cp.tile([128, S], BF16)
            nc.scalar.dma_start(out=cos_s[:, :], in_=cosT[:, :])
            csin_s = cp.tile([128, S], BF16)
            nc.scalar.dma_start(out=csin_s[:, :], in_=csinT[:, :])
            qw_s = cp.tile([128, 1], F32)
            nc.scalar.dma_start(out=qw_s[:, :], in_=qw[:, :])
            kw_s = cp.tile([128, 1], F32)
            nc.scalar.dma_start(out=kw_s[:, :], in_=kw[:, :])

            a2a0_in = dramp.tile([H * D, TCB], BF16, name="a2a0i")
            a2a0_out = dramp.tile([H * D, TCB], BF16, name="a2a0o")
            a2a1_in = [dramp.tile([cores * HH * 128, TCB], BF16,
                                  tag=f"a2a1i{p}", name=f"a2a1i{p}")
                       for p in range(2)]
            a2a1_out = [dramp.tile([cores * HH * 128, TCB], BF16,
                                   tag=f"a2a1o{p}", name=f"a2a1o{p}")
                        for p in range(2)]

            qT_s = p_qkv.tile([128, HQ * T], BF16, tag="qT")
            kT_s = p_qkv.tile([128, T], BF16, tag="kT")
            vnat_s = p_qkv.tile([128, T], BF16, tag="vnat")
            ctxT_s = p_qkv.tile([128, HQ * T], BF16, tag="ctxT")

            def rsqrt_act(out_ap, in_ap, bias_ap, sc):
                """rs = rsqrt(in*sc + bias) on ACT.  reciprocal_sqrt lives in
                a table set that also has square/copy, so the whole proj
                phase stays on one set (no ~1.3us reloads per tile)."""
                eng = nc.scalar
                ins = [eng.lower_ap(in_ap), eng.lower_ap(bias_ap),
                       mybir.ImmediateValue(dtype=mybir.dt.float32,
                                            value=float(sc)),
                       mybir.ImmediateValue(dtype=mybir.dt.float32,
                                            value=0.0)]
                return eng.add_instruction(
                    mybir.InstActivation(
                        name=nc.get_next_instruction_name(),
                        func=mybir.ActivationFunctionType.Rsqrt,
                        ins=ins, outs=[eng.lower_ap(out_ap)]))

            def proj(b, wts, p_hid, p_pw, wload_hook=None):
                """QKV projection + norm + rope for batch b (tt-outer)."""
                for tt in range(TPB):
                    tg = b * S + tt * TT
                    pos = tt * TT
                    hts = []
                    for k in range(NHT):
                        t_ = p_hid.tile([128, HB, TT], BF16, tag="hid",
                                        name="hid")
                        nc.sync.dma_start(
                            out=t_[:, :, :],
                            in_=hT[b, tt, k * HB:(k + 1) * HB]
                            .rearrange("c p s -> p c s"))
                        hts.append(t_)
                        if wload_hook is not None and tt == 0 and k < HQ + 1:
                            wload_hook(k + 1)
                    for ob in range(HQ + 2):
                        w_t = wts[ob]
                        ps = ps_all.tile([128, TT], F32, tag="mm", name="ps",
                                         bufs=2, padded_shape=[128, SW])
                        for ch in range(HCH):
                            nc.tensor.matmul(
                                ps[:, :],
                                lhsT=w_t[:, ch * 128:(ch + 1) * 128],
                                rhs=hts[ch // HB][:, ch % HB, :],
                                start=(ch == 0), stop=(ch == HCH - 1))
                        if ob <= HQ:
                            is_q = ob < HQ
                            dst = (qT_s[:, ob * T + tg: ob * T + tg + TT]
                                   if is_q else kT_s[:, tg: tg + TT])
                            wcol = qw_s if is_q else kw_s
                            sq = p_pw.tile([128, TT], BF16, tag="sq")
                            nc.scalar.square(sq[:, :], ps[:, :])
                            ssq = ps_all.tile([128, TT], F32, tag="aux",
                                              name="ssq", bufs=2)
                            nc.tensor.matmul(ssq[:, :], lhsT=ones_s[:, :],
                                             rhs=sq[:, :], start=True,
                                             stop=True)
                            rs = p_pw.tile([128, TT], F32, tag="rs")
                            rsqrt_act(rs[:, :], ssq[:, :], eps_s[:, :],
                                      1.0 / D)
                            qn = p_pw.tile([128, TT], F32, tag="qn")
                            nc.vector.scalar_tensor_tensor(
                                qn[:, :], in0=ps[:, :], scalar=wcol[:, :],
                                in1=rs[:, :], op0=MULT, op1=MULT)
                            qsw = p_pw.tile([128, TT], F32, tag="qsw")
                            nc.scalar.dma_start(out=qsw[0:64, :],
                                                in_=qn[64:128, :])
                            nc.scalar.dma_start(out=qsw[64:128, :],
                                                in_=qn[0:64, :])
                            t1 = p_pw.tile([128, TT], F32, tag="t1")
                            nc.vector.tensor_mul(t1[:, :], qn[:, :],
                                                 cos_s[:, pos: pos + TT])
                            t2 = p_pw.tile([128, TT], BF16, tag="t2")
                            nc.vector.tensor_mul(t2[:, :], qsw[:, :],
                                                 csin_s[:, pos: pos + TT])
                            nc.vector.tensor_add(dst, t1[:, :], t2[:, :])
                        else:
                            vt = p_pw.tile([128, TT], BF16, tag="vt")
                            nc.scalar.copy(vt[:, :], ps[:, :])
                            for tb in range(TT // 128):
                                vtr = ps_all.tile([128, 128], BF16, tag="aux",
                                                  name="vtr", bufs=2)
                                nc.tensor.transpose(
                                    vtr[:, :], vt[:, tb * 128:(tb + 1) * 128],
                                    ident[:, :])
                                tbg = tg // 128 + tb
                                nc.scalar.copy(
                                    vnat_s[:, tbg * 128:(tbg + 1) * 128],
                                    vtr[:, :])

            def attn(b, mid_hook=None):
                """Attention for batch b + context shipping (A2A)."""
                last = b == B - 1
                for h in range(HQ):
                    qoff = h * T + b * S
                    pt_t = p_pt.tile([128, KB * SW], BF16, tag="pT",
                                     name="pT")
                    for kb in range(KB):
                        sps = ps_all.tile([128, SW], F32, tag="mm",
                                          name="sps", bufs=2)
                        for qt in range(QTB):
                            nc.tensor.matmul(
                                sps[:, qt * QT:(qt + 1) * QT],
                                lhsT=kT_s[:, b * S + kb * 128:
                                          b * S + (kb + 1) * 128],
                                rhs=qT_s[:, qoff + qt * QT:
                                         qoff + (qt + 1) * QT],
                                start=True, stop=True)
                        nc.scalar.activation(
                            pt_t[:, kb * SW:(kb + 1) * SW], sps[:, :],
                            mybir.ActivationFunctionType.Exp, scale=scale)
                    ctxs = [ps_all.tile([128, QT], F32, tag="ctx", name="ctx",
                                        bufs=2) for _ in range(QTB)]
                    for kb in range(KB):
                        tbg = (b * S) // 128 + kb
                        for qt in range(QTB):
                            nc.tensor.matmul(
                                ctxs[qt][:, :],
                                lhsT=vnat_s[:, tbg * 128:(tbg + 1) * 128],
                                rhs=pt_t[:, kb * SW + qt * QT:
                                         kb * SW + (qt + 1) * QT],
                                start=(kb == 0), stop=(kb == KB - 1))
                    denp = p_aw.tile([128, SW], BF16, tag="denp")
                    dent = p_aw.tile([128, SW], BF16, tag="dent")
                    assert KB % 4 == 0
                    nc.vector.tensor_add(denp[:, :], pt_t[:, 0:SW],
                                         pt_t[:, SW:2 * SW])
                    nc.vector.tensor_add(dent[:, :],
                                         pt_t[:, 2 * SW:3 * SW],
                                         pt_t[:, 3 * SW:4 * SW])
                    nc.vector.tensor_add(denp[:, :], denp[:, :], dent[:, :])
                    for g in range(1, KB // 4):
                        nc.vector.tensor_add(
                            dent[:, :],
                            pt_t[:, 4 * g * SW:(4 * g + 1) * SW],
                            pt_t[:, (4 * g + 1) * SW:(4 * g + 2) * SW])
                        nc.vector.tensor_add(denp[:, :], denp[:, :],
                                             dent[:, :])
                        nc.vector.tensor_add(
                            dent[:, :],
                            pt_t[:, (4 * g + 2) * SW:(4 * g + 3) * SW],
                            pt_t[:, (4 * g + 3) * SW:(4 * g + 4) * SW])
                        nc.vector.tensor_add(denp[:, :], denp[:, :],
                                             dent[:, :])
                    for qt in range(QTB):
                        dps = ps_all.tile([128, QT], F32, tag="aux",
                                          name="dps", bufs=2)
                        nc.tensor.matmul(dps[:, :], lhsT=ones_s[:, :],
                                         rhs=denp[:, qt * QT:(qt + 1) * QT],
                                         start=True, stop=True)
                        rec = p_aw.tile([128, QT], F32, tag="rec")
                        nc.vector.reciprocal_approx_fast(rec[:, :], dps[:, :])
                        nc.vector.tensor_mul(
                            ctxT_s[:, qoff + qt * QT: qoff + (qt + 1) * QT],
                            ctxs[qt][:, :], rec[:, :])
                    # ship this head's context (one batched DMA per head)
                    if not last:
                        nc.gpsimd.dma_start(
                            out=a2a0_in[:, :].rearrange(
                                "(j q p) t -> q p j t", j=cores, p=128)[h],
                            in_=ctxT_s[:, qoff: qoff + S].rearrange(
                                "p (j t) -> p j t", j=cores))
                    else:
                        pi, hh = h // HH, h % HH
                        nc.gpsimd.dma_start(
                            out=a2a1_in[pi][:, :].rearrange(
                                "(j q p) t -> q p j t", j=cores, p=128)[hh],
                            in_=ctxT_s[:, qoff: qoff + S].rearrange(
                                "p (j t) -> p j t", j=cores))
                        if hh == HH - 1:
                            nc.gpsimd.collective_compute(
                                "AllToAll", mybir.AluOpType.bypass,
                                replica_groups=[list(range(cores))],
                                ins=[a2a1_in[pi].opt()],
                                outs=[a2a1_out[pi].opt()])
                            if pi == 0 and mid_hook is not None:
                                mid_hook()
                if not last:
                    nc.gpsimd.collective_compute(
                        "AllToAll", mybir.AluOpType.bypass,
                        replica_groups=[list(range(cores))],
                        ins=[a2a0_in.opt()],
                        outs=[a2a0_out.opt()])

            # ---- phase 1: proj0, attn0 (+a2a0), proj1 ----
            with (
                tc.tile_pool(name="hid", bufs=10) as p_hid,
                tc.tile_pool(name="wts", bufs=6) as p_w,
                tc.tile_pool(name="pwork", bufs=2) as p_pw,
            ):
                # QKV weights: load once, reused for both batches.  Emit
                # interleaved with the first hidden pass so ob=1.. aren't
                # gated behind all of hT.
                wts = {}

                def wload(ob):
                    w_t = p_w.tile([128, HCH * 128], BF16, tag="w", name="w")
                    src = (wq[ob] if ob < HQ else
                           (wk[:, :] if ob == HQ else wv[:, :]))
                    nc.sync.dma_start(out=w_t[:, :], in_=src)
                    wts[ob] = w_t

                wload(0)
                sc_ = nc.enter_named_scope("proj0", True)[0]
                proj(0, wts, p_hid, p_pw, wload_hook=wload)
                nc.leave_named_scope("proj0", sc_, True)
                sc_ = nc.enter_named_scope("attn0", True)[0]
                attn(0)
                nc.leave_named_scope("attn0", sc_, True)
                sc_ = nc.enter_named_scope("proj1", True)[0]
                proj(1, wts, p_hid, p_pw)
                nc.leave_named_scope("proj1", sc_, True)

            # ---- phase 2: attn1 (split a2a) + o_proj ----
            with (
                tc.tile_pool(name="wo", bufs=11) as p_wo,
                tc.tile_pool(name="cx", bufs=1) as p_cx,
                tc.tile_pool(name="oo", bufs=4) as p_oo,
            ):
                wo_tiles = {}   # (grp, k) -> tile

                def load_wo_tile(g, k):
                    w_t = p_wo.tile([128, WB, GSZ * OH], BF16, tag="wo",
                                    name="wo")
                    nc.sync.dma_start(
                        out=w_t[:, :, :],
                        in_=wo[g, k * WB:(k + 1) * WB]
                        .rearrange("c p w -> p c w"))
                    wo_tiles[(g, k)] = w_t

                def wo_view(g, ic, i):
                    w_t = wo_tiles[(g, ic // WB)]
                    return w_t[:, ic % WB, i * OH:(i + 1) * OH]

                # prefetch grp0 before attn1 (transfers overlap attn1)
                for k in range(NWT):
                    load_wo_tile(0, k)

                cx0_s = p_cx.tile([128, ICH * TCB], BF16, tag="cx0",
                                  name="cx0")
                # cx1 split per a2a half so b1 o_proj can start on the
                # first half while the second collective is still in flight
                cx1h = [p_cx.tile([128, (ICH // 2) * TCB], BF16,
                                  tag=f"cx1h{p}", name=f"cx1h{p}")
                        for p in range(2)]
                # cx0: a2a0 done long ago; scalar queue
                nc.scalar.dma_start(
                    out=cx0_s[:, :].rearrange("p (ic t) -> p ic t", ic=ICH),
                    in_=a2a0_out[:, :].rearrange("(ic p) t -> p ic t",
                                                 ic=ICH))

                def _mid_hook():
                    for k in range(4):
                        load_wo_tile(1, k)

                def cx_ap(b, ic):
                    if b == 0:
                        return cx0_s[:, ic * TCB:(ic + 1) * TCB]
                    # ic = global head = j * HQ + (pi * HH + hh)
                    q = ic % HQ
                    j = ic // HQ
                    pi, hh = q // HH, q % HH
                    off = (hh * cores + j) * TCB
                    return cx1h[pi][:, off: off + TCB]

                # b=1 accumulation order: pi0's chunks first (they land
                # ~20us before pi1's)
                IC_B1 = [j * HQ + pi * HH + hh
                         for pi in range(2) for hh in range(HH)
                         for j in range(cores)]

                sc_ = nc.enter_named_scope("attn1", True)[0]
                attn(1, mid_hook=_mid_hook)
                nc.leave_named_scope("attn1", sc_, True)

                # cx1 gather on scalar queue (waits on a2a1; does not block
                # the sync queue where wo loads live)
                for pi in range(2):
                    srcv = a2a1_out[pi][:, :].rearrange(
                        "(j hh p) t -> p hh j t", hh=HH, p=128)
                    dstv = cx1h[pi][:, :].rearrange(
                        "p (hh j t) -> p hh j t", hh=HH, j=cores)
                    for hh in range(HH):
                        nc.scalar.dma_start(
                            out=dstv[:, hh],
                            in_=srcv[:, hh])

                sc_ = nc.enter_named_scope("oproj", True)[0]

                def pso_mms(pso_t, b, g, i, k0, k1):
                    order = IC_B1 if b == 1 else list(range(ICH))
                    for kk in range(k0, k1):
                        ic = order[kk]
                        nc.tensor.matmul(
                            pso_t[:, :],
                            lhsT=cx_ap(b, ic),
                            rhs=wo_view(g, ic, i),
                            start=(kk == 0), stop=(kk == ICH - 1))

                def finish(pso_t, b, g, i):
                    hg = g * GSZ + i
                    ot = p_oo.tile([TCB, OH], F32, tag="oout", name="oout")
                    nc.vector.tensor_copy(ot[:, :], pso_t[:, :])
                    nc.sync.dma_start(
                        out=out[b * TCB:(b + 1) * TCB,
                                hg * OH:(hg + 1) * OH],
                        in_=ot[:, :])

                def full_block(b, g, i, tag="aux"):
                    pso_t = ps_all.tile([TCB, OH], F32, tag=tag, name="pso",
                                        bufs=2)
                    pso_mms(pso_t, b, g, i, 0, ICH)
                    finish(pso_t, b, g, i)

                # b0:g0 (wo prefetched, cx0 ready)
                full_block(0, 0, 0)
                full_block(0, 0, 1)
                # filler: partial b0:g1 with the 4 prefetched g1 tiles
                pC = ps_all.tile([TCB, OH], F32, tag="ctx", name="psoC",
                                 bufs=2)
                pso_mms(pC, 0, 1, 0, 0, 4 * WB)
                pD = ps_all.tile([TCB, OH], F32, tag="ctx", name="psoD",
                                 bufs=2)
                pso_mms(pD, 0, 1, 1, 0, 4 * WB)
                # b1:g0 half-interleaved: first halves need only the
                # pi0 a2a half (lands ~20us before pi1)
                pE = ps_all.tile([TCB, OH], F32, tag="aux", name="psoE",
                                 bufs=2)
                pF = ps_all.tile([TCB, OH], F32, tag="aux", name="psoF",
                                 bufs=2)
                pso_mms(pE, 1, 0, 0, 0, ICH // 2)
                pso_mms(pF, 1, 0, 1, 0, ICH // 2)
                pso_mms(pE, 1, 0, 0, ICH // 2, ICH)
                finish(pE, 1, 0, 0)
                pso_mms(pF, 1, 0, 1, ICH // 2, ICH)
                finish(pF, 1, 0, 1)
                # rest of g1 + all of g2 stream into slots freed by b1:g0
                for k in range(4, NWT):
                    load_wo_tile(1, k)
                for k in range(NWT):
                    load_wo_tile(2, k)
                pso_mms(pC, 0, 1, 0, 4 * WB, ICH)
                finish(pC, 0, 1, 0)
                pso_mms(pD, 0, 1, 1, 4 * WB, ICH)
                finish(pD, 0, 1, 1)
                full_block(1, 1, 0)
                full_block(1, 1, 1)
                full_block(0, 2, 0)
                full_block(0, 2, 1)
                for k in range(NWT):
                    load_wo_tile(3, k)
                full_block(1, 2, 0)
                full_block(1, 2, 1)
                for b in range(B):
                    full_block(b, 3, 0)
                    full_block(b, 3, 1)
                nc.leave_named_scope("oproj", sc_, True)

    nc.compile()
    return nc


def host_prep(inputs, B=2, S=1024, HID=4096, H=32, KV=8, D=128, eps=1e-6):
    """Shard + lay out the full inputs into per-core in_maps."""
    cores = N_CORES
    HQ = H // cores
    T = B * S
    HCH = HID // 128
    ICH = (H * D) // 128

    hs = np.ascontiguousarray(inputs["hidden_states"], dtype=np.float32)
    fc = np.asarray(inputs["freqs_cis"], dtype=np.float32)
    Wq = np.asarray(inputs["Wq"], dtype=np.float32)
    Wk = np.asarray(inputs["Wk"], dtype=np.float32)
    Wv = np.asarray(inputs["Wv"], dtype=np.float32)
    Wo = np.asarray(inputs["Wo"], dtype=np.float32)
    qnw = np.asarray(inputs["q_norm_w"], dtype=np.float32)
    knw = np.asarray(inputs["k_norm_w"], dtype=np.float32)

    # hidden^T chunks, tt-major: hT[b, tt, ch, p, s] = hs[b, tt*TT+s, ch*128+p]
    TT = 512
    TPB = S // TT
    hT = np.ascontiguousarray(
        hs.transpose(0, 2, 1).reshape(B, HCH, 128, TPB, TT)
        .transpose(0, 3, 1, 2, 4)).astype(BF16_NP)

    cos, sin, nsin = fc[0], fc[1], fc[2]      # [S, D]
    cosT = np.ascontiguousarray(cos.T).astype(BF16_NP)    # [128, S]
    csinT = np.concatenate([nsin.T[0:64], sin.T[64:128]], axis=0)
    csinT = np.ascontiguousarray(csinT).astype(BF16_NP)
    qw_col = np.ascontiguousarray(qnw.reshape(128, 1))
    kw_col = np.ascontiguousarray(knw.reshape(128, 1))

    # Wo^T chunks, group-major: wo[g, ic, p, w] = Wo[g*1024+w, ic*128+p]
    woT = np.ascontiguousarray(
        Wo.T.reshape(ICH, 128, 4, 1024).transpose(2, 0, 1, 3)) \
        .astype(BF16_NP)

    def prep_w(Wm, nblocks):
        # [nblocks, p, ch*128] with w[ob, p, ch*128+j] = Wm[ob*128+j, ch*128+p]
        a = Wm.reshape(nblocks, 128, HCH, 128).transpose(0, 3, 2, 1)
        return np.ascontiguousarray(a.reshape(nblocks, 128, HCH * 128)) \
            .astype(BF16_NP)

    in_maps = []
    for c in range(cores):
        Wq_c = Wq[c * HQ * D:(c + 1) * HQ * D]
        Wk_c = Wk[c * D:(c + 1) * D]
        Wv_c = Wv[c * D:(c + 1) * D]
        in_maps.append({
            "hT": hT,
            "wq": prep_w(Wq_c, HQ),
            "wk": prep_w(Wk_c, 1)[0],
            "wv": prep_w(Wv_c, 1)[0],
            "wo": woT,
            "cosT": cosT,
            "csinT": csinT,
            "qw": qw_col,
            "kw": kw_col,
        })
    return in_maps


def gather_output(results, B=2, S=1024, HID=4096, **_):
    cores = N_CORES
    TCB = (B * S) // cores // B
    out = np.empty((B, S, HID), dtype=np.float32)
    for c in range(cores):
        o = results[c]["out"]
        for b in range(B):
            out[b, c * TCB:(c + 1) * TCB] = o[b * TCB:(b + 1) * TCB]
    return out


_NC_CACHE = {}


def kernel(**inputs) -> np.ndarray:
    cfg = FULL_CFG
    key = tuple(sorted(cfg.items()))
    if key not in _NC_CACHE:
        _NC_CACHE[key] = build_program(**cfg)
    nc = _NC_CACHE[key]
    in_maps = host_prep(inputs, **cfg)
    res = run_bass_kernel_spmd(nc, in_maps, core_ids=list(range(N_CORES)))
    return gather_output(res.results, **cfg)
